# revision 4
# baseline (speedup 1.0000x reference)
"""Trainium2 Bass kernel: per-batch spatial self-attention (fp8/bf16 mixed).

Per-core (one batch image per NeuronCore, 8 cores):
  Projections q,k,v in float32r (tf32) from f32 x — exact bases.
  q8 = e4m3(q); dq = q - q8 kept in f32r.  k8 = e4m3(k).  v in bf16.
  bq enters as the per-key scalar (bq.k8_m)/16 folded into the exp bias;
  bk is softmax-invariant and dropped.

  Scores St[m,n] = k8^T q8 — ONE fp8 DoubleRow matmul per 128-key chunk
     (contracts all 256 channels at the fp8 rate).
  The q8 quantization error couples to the output through cov(k,v); it is
  cancelled by a rank-256 correction accumulated straight into the
  attention output PSUM:  OT += dq^T (Wk^T Wv) * (NPIX*chat/16)  — the
  `om` input, with chat = E[P] estimated host-side by sampling.
  P = exp(St/16 + bqk) in bf16: Act-engine exp for 24/32 key chunks, the
  bf16 Schraudolph bit-trick on DVE for 8/32.
  OT = v^T P in bf16; rowsums via Pool-engine accumulators (SBUF only —
  GPSIMD cannot touch PSUM) + a ones-matmul over partitions.
"""

import sys

sys.path.insert(0, "/opt/trn_rl_repo")

import numpy as np
import concourse.bacc as bacc
import concourse.mybir as mybir
import concourse.tile as tile
from concourse.bass_utils import run_bass_kernel_spmd

F32 = mybir.dt.float32
F32R = mybir.dt.float32r
FP8 = mybir.dt.float8e4
BF16 = mybir.dt.bfloat16
I16 = mybir.dt.int16
AF = mybir.ActivationFunctionType
ALU = mybir.AluOpType
DRM = mybir.MatmulPerfMode.DoubleRow

B = 8
C = 256
NPIX = 4096
NT = NPIX // 512
M = NPIX // 128
PAIRS = M // 2
SCALE = 1.0 / 16.0
LAG_P = 7
# exp engine per key-chunk (32 halves/nt): a=Act exp, d=DVE bf16-Schraudolph
# (bits16 = y*128*log2e + 126.946*128, truncating convert)
EPAT = ["a", "a", "a", "d", "a", "a", "a", "d",
        "a", "a", "d", "a", "a", "a", "a", "d"] * 2
A16 = 128.0 * np.log2(np.e)
B16 = 126.946 * 128.0

_CACHE = {}


def _build():
    nc = bacc.Bacc("TRN2", num_swdge_queues=4)
    x_d = nc.declare_dram_parameter("x", [C, NPIX], F32, isOutput=False)
    wq_d = nc.declare_dram_parameter("wq_t", [C, C], F32, isOutput=False)
    wk_d = nc.declare_dram_parameter("wk_t", [C, C], F32, isOutput=False)
    wv_d = nc.declare_dram_parameter("wv_t", [C, C], F32, isOutput=False)
    om_d = nc.declare_dram_parameter("om", [C, C], F32, isOutput=False)
    bq_d = nc.declare_dram_parameter("bq", [C, 1], F32, isOutput=False)
    bv_d = nc.declare_dram_parameter("bv", [1, C], F32, isOutput=False)
    out_d = nc.declare_dram_parameter("out", [C, NPIX], F32, isOutput=True)

    with tile.TileContext(nc) as tc:
        with (
            tc.tile_pool(name="big", bufs=1) as big,
            tc.tile_pool(name="small", bufs=2) as small,
            tc.tile_pool(name="ptp", bufs=LAG_P + 3) as ptp,
            tc.tile_pool(name="outp", bufs=4) as outp,
            tc.tile_pool(name="psS", bufs=2, space="PSUM") as psS,
            tc.tile_pool(name="psO", bufs=1, space="PSUM") as psO,
            tc.tile_pool(name="psW", bufs=1, space="PSUM") as psW,
            tc.tile_pool(name="psR", bufs=1, space="PSUM") as psR,
        ):
            # ---- input DMAs (need-ordered) ----
            w_r = {}
            for nm in ("q", "k", "v", "o"):
                w_r[nm] = [
                    big.tile([128, C], F32R, name=f"w{nm}_r{i}") for i in range(2)
                ]
            for nm, wd in (("q", wq_d), ("k", wk_d)):
                for i in range(2):
                    nc.gpsimd.dma_start(
                        out=w_r[nm][i], in_=wd[i * 128 : (i + 1) * 128, :]
                    )
            x_r = [big.tile([128, NPIX], F32R, name=f"x_r{i}") for i in range(2)]
            for j in range(2):
                lo, hi = j * 512, (j + 1) * 512
                for i in range(2):
                    nc.gpsimd.dma_start(
                        out=x_r[i][:, lo:hi], in_=x_d[i * 128 : (i + 1) * 128, lo:hi]
                    )
            for nm, wd in (("v", wv_d), ("o", om_d)):
                for i in range(2):
                    nc.gpsimd.dma_start(
                        out=w_r[nm][i], in_=wd[i * 128 : (i + 1) * 128, :]
                    )
            for j in range(2, 8):
                lo, hi = j * 512, (j + 1) * 512
                for i in range(2):
                    nc.gpsimd.dma_start(
                        out=x_r[i][:, lo:hi], in_=x_d[i * 128 : (i + 1) * 128, lo:hi]
                    )
            bq8 = big.tile([128, 2, 1], FP8, name="bq8")
            for i in range(2):
                nc.gpsimd.dma_start(
                    out=bq8[:, i, :], in_=bq_d[i * 128 : (i + 1) * 128, :]
                )
            bv_f = big.tile([1, C], F32, name="bv_f")
            nc.sync.dma_start(out=bv_f, in_=bv_d[:, :])
            bv_r = big.tile([1, C], F32R, name="bv_r")
            nc.vector.tensor_copy(bv_r, bv_f)

            ones_f = big.tile([128, 1], F32, name="ones_f")
            nc.vector.memset(ones_f, 1.0)
            ones_rf = big.tile([1, 128], F32, name="ones_rf")
            nc.vector.memset(ones_rf, 1.0)
            ones_colr = big.tile([1, 128], F32R, name="ones_colr")
            nc.vector.tensor_copy(ones_colr, ones_rf)
            ones_row = big.tile([1, 128], F32R, name="ones_row")
            nc.vector.tensor_copy(ones_row, ones_rf)
            ones_col = big.tile([128, 1], F32R, name="ones_col")
            nc.vector.tensor_copy(ones_col, ones_f)

            # ---- PE warmup (p-state ramp), baseline pattern ----
            warm_f = small.tile([128, 256], F32, name="warm_f", tag="warm_f")
            nc.vector.memset(warm_f, 1.0)
            warm_r = small.tile([128, 256], F32R, name="warm_r", tag="warm_r")
            nc.vector.tensor_copy(warm_r, warm_f)
            warm_ps = psR.tile([1, 256], F32, name="warm_ps", tag="psR")
            for _ in range(48):
                nc.tensor.matmul(
                    warm_ps, ones_col, warm_r, start=True, stop=True,
                    skip_group_check=True,
                )

            # ---- Q/K projections (f32r), nt-major pair layout ----
            q8 = big.tile([128, NT, 2, 512], FP8, name="q8")
            dq_r = big.tile([128, NT, 2, 512], F32R, name="dq_r")
            k8 = big.tile([128, NT, 2, 512], FP8, name="k8")
            for nt in range(NT):
                sl = slice(nt * 512, (nt + 1) * 512)
                for wkey in ("q", "k"):
                    ps = psS.tile([128, 1024], F32, name="ps_proj", tag="pair")
                    for o in range(2):
                        for i in range(2):
                            nc.tensor.matmul(
                                ps[:, o * 512 : (o + 1) * 512],
                                w_r[wkey][i][:, o * 128 : (o + 1) * 128],
                                x_r[i][:, sl],
                                start=(i == 0),
                                stop=(i == 1),
                            )
                    if wkey == "k":
                        nc.scalar.activation(k8[:, nt], ps, AF.Copy)
                    else:
                        if nt % 2 == 0:
                            nc.vector.tensor_copy(q8[:, nt], ps)
                        else:
                            nc.scalar.activation(q8[:, nt], ps, AF.Copy)
                        nc.vector.tensor_tensor(
                            dq_r[:, nt], ps, q8[:, nt], ALU.subtract
                        )

            # ---- bqk[m] = (bq . k8_m) (exp bias), tiny DR matmuls ----
            def k8_lhsT(m):
                return k8[:, m // 4, :, (m % 4) * 128 : (m % 4 + 1) * 128]

            bqk_ps = psR.tile([128, M], F32, name="bqk_ps", tag="psR")
            for m in range(M):
                nc.tensor.matmul(
                    bqk_ps[:, m : m + 1], k8_lhsT(m), bq8,
                    start=True, stop=True, perf_mode=DRM,
                )
            s2a = big.tile([128, M], F32, name="s2a")
            nc.vector.tensor_scalar(s2a, bqk_ps, SCALE, None, ALU.mult)
            s2d = big.tile([128, M], F32, name="s2d")
            nc.vector.tensor_scalar(s2d, bqk_ps, A16 * SCALE, B16,
                                    ALU.mult, ALU.add)

            # ---- V projection (f32r, bias via ones-matmul) -> bf16 ----
            v_bf = big.tile([128, M, C], BF16, name="v_bf")
            for g in range(8):  # 4 m-chunks per psum pair tile
                ps = psS.tile([128, 1024], F32, name="ps_v", tag="pair")
                for j in range(4):
                    m = 4 * g + j
                    po = ps[:, j * 256 : (j + 1) * 256]
                    for i in range(2):
                        nc.tensor.matmul(
                            po,
                            x_r[i][:, m * 128 : (m + 1) * 128],
                            w_r["v"][i],
                            start=(i == 0),
                            stop=False,
                        )
                    nc.tensor.matmul(po, ones_colr, bv_r, start=False, stop=True)
                for j in range(2):  # two [128,512] pair copies
                    m = 4 * g + 2 * j
                    po = ps[:, j * 512 : (j + 1) * 512]
                    if j == 0:
                        nc.scalar.activation(v_bf[:, m : m + 2, :], po, AF.Copy)
                    else:
                        nc.vector.tensor_copy(v_bf[:, m : m + 2, :], po)

            # ---- attention ----
            def drain_a(state):
                """rowsum -> reciprocal (off the PE critical path)."""
                p_ot0, p_ot1, p_accD, p_accP, p_nt = state
                rs = psW.tile([1, 512], F32, name="rs", tag="rs")
                nc.tensor.matmul(rs, ones_col, p_accD, start=True, stop=False)
                nc.tensor.matmul(rs, ones_col, p_accP, start=False, stop=True)
                rinv = small.tile([1, 512], F32, name="rinv", tag="rinv")
                nc.vector.reciprocal_approx_fast(rinv, rs)
                rinv_r = small.tile([1, 512], F32R, name="rinv_r", tag="rinv_r")
                nc.gpsimd.tensor_copy(rinv_r, rinv)
                return rinv_r

            def drain_b(state, rinv_r):
                """broadcast 1/rowsum, normalize, store."""
                p_ot0, p_ot1, p_accD, p_accP, p_nt = state
                rb = psR.tile([128, 512], F32, name="rb", tag="psR")
                nc.tensor.matmul(rb, ones_row, rinv_r, start=True, stop=True)
                rb_sb = small.tile([128, 512], F32, name="rb_sb", tag="rb_sb")
                nc.scalar.activation(rb_sb, rb, AF.Copy)
                psl = slice(p_nt * 512, (p_nt + 1) * 512)
                for o, ot in enumerate((p_ot0, p_ot1)):
                    osb = outp.tile([128, 512], F32, name="osb", tag="osb")
                    nc.vector.tensor_tensor(osb, ot, rb_sb, ALU.mult)
                    nc.sync.dma_start(
                        out=out_d[o * 128 : (o + 1) * 128, psl], in_=osb
                    )

            prev = None
            rinv_prev = None
            for nt in range(NT):
                ot0 = psO.tile([128, 512], F32, name="ot0", tag="ot0")
                ot1 = psO.tile([128, 512], F32, name="ot1", tag="ot1")
                accD = small.tile([128, 512], F32R, name="accD", tag="accD")
                accP = small.tile([128, 512], F32R, name="accP", tag="accP")
                pts = {}
                for pp in range(PAIRS + LAG_P):
                    if pp < PAIRS:
                        st = psS.tile([128, 1024], F32, name="st", tag="pair")
                        pt = ptp.tile([128, 2, 512], BF16, name="pt")
                        for h in range(2):
                            m = 2 * pp + h
                            half = st[:, h * 512 : (h + 1) * 512]
                            nc.tensor.matmul(
                                half, k8_lhsT(m), q8[:, nt],
                                start=True, stop=True, perf_mode=DRM,
                            )
                            if EPAT[m] == "a":
                                nc.scalar.activation(
                                    pt[:, h, :], half, AF.Exp,
                                    bias=s2a[:, m : m + 1], scale=SCALE,
                                )
                            else:
                                nc.vector.tensor_scalar(
                                    pt[:, h, :].bitcast(I16), half,
                                    A16 * SCALE, s2d[:, m : m + 1],
                                    ALU.mult, ALU.add,
                                )
                            # rowsum accumulators live on Pool (SBUF-only)
                            acc = accD if h == 0 else accP
                            if pp == 0:
                                nc.gpsimd.tensor_copy(acc, pt[:, h, :])
                            else:
                                nc.gpsimd.tensor_tensor(
                                    acc, acc, pt[:, h, :], ALU.add
                                )
                        pts[pp] = pt
                    if prev is not None and pp == 1:
                        rinv_prev = drain_a(prev)
                    if prev is not None and pp == 5:
                        drain_b(prev, rinv_prev)
                    if pp >= LAG_P:
                        p = pp - LAG_P
                        pt = pts.pop(p)
                        for h in range(2):
                            m = 2 * p + h
                            rhs = pt[:, h, :]
                            first = (p == 0 and h == 0)
                            last = (p == PAIRS - 1 and h == 1)
                            nc.tensor.matmul(
                                ot0, v_bf[:, m, 0:128], rhs,
                                start=first, stop=last,
                            )
                            nc.tensor.matmul(
                                ot1, v_bf[:, m, 128:256], rhs,
                                start=first, stop=last,
                            )
                        # omega correction joins each psum group mid-way
                        if p == 2:
                            for o, ot in enumerate((ot0, ot1)):
                                for i in range(2):
                                    nc.tensor.matmul(
                                        ot,
                                        w_r["o"][i][:, o * 128 : (o + 1) * 128],
                                        dq_r[:, nt, i, :],
                                        start=False,
                                        stop=False,
                                    )
                prev = (ot0, ot1, accD, accP, nt)

            rinv_prev = drain_a(prev)
            drain_b(prev, rinv_prev)

    nc.compile()
    return nc


def _get_nc():
    if "nc" not in _CACHE:
        _CACHE["nc"] = _build()
    return _CACHE["nc"]


def _host_prep(x, wq, wk, wv, bq, bv):
    """Per-core input maps incl. the omega correction matrix."""
    Wq = np.ascontiguousarray(wq.T)
    Wk = np.ascontiguousarray(wk.T)
    Wv = np.ascontiguousarray(wv.T)
    om_base = (wk @ wv.T).astype(np.float32)  # = Wk.T @ Wv
    rng = np.random.default_rng(12345)
    shared = {
        "wq_t": Wq,
        "wk_t": Wk,
        "wv_t": Wv,
        "bq": np.ascontiguousarray(bq.reshape(C, 1)),
        "bv": np.ascontiguousarray(bv.reshape(1, C)),
    }
    maps = []
    for b in range(B):
        xb = np.ascontiguousarray(x[b].reshape(C, NPIX))
        qi = rng.choice(NPIX, 128, replace=False)
        ki = rng.choice(NPIX, 256, replace=False)
        qs = xb[:, qi].T @ Wq  # unbiased q; bias enters via bqk
        ks = xb[:, ki].T @ Wk
        bqks = ks @ bq.reshape(-1)
        y = (qs @ ks.T) * SCALE + bqks[None, :] * SCALE
        chat = float(np.exp(y).mean())
        om = np.ascontiguousarray(om_base * (NPIX * chat / 16.0))
        maps.append({"x": xb, "om": om, **shared})
    return maps


def kernel(x, wq, wk, wv, bq, bk, bv):
    x = np.asarray(x, dtype=np.float32)
    wq = np.asarray(wq, dtype=np.float32)
    wk = np.asarray(wk, dtype=np.float32)
    wv = np.asarray(wv, dtype=np.float32)
    bq = np.asarray(bq, dtype=np.float32)
    bv = np.asarray(bv, dtype=np.float32)
    nc = _get_nc()
    res = run_bass_kernel_spmd(
        nc, _host_prep(x, wq, wk, wv, bq, bv), core_ids=list(range(B))
    )
    out = np.stack([res.results[b]["out"] for b in range(B)])
    return out.reshape(B, C, 64, 64)


# revision 5
# speedup vs baseline: 1.3060x; 1.3060x over previous
"""Trainium2 Bass kernel: per-batch spatial self-attention (fp8/bf16 mixed).

Per-core (one batch image per NeuronCore, 8 cores):
  Projections q,k,v in float32r (tf32) from f32 x — exact bases.
  q8 = e4m3(q); dq = q - q8 kept in f32r.  k8 = e4m3(k).  v in bf16.
  bq enters as the per-key scalar (bq.k8_m)/16 folded into the exp bias;
  bk is softmax-invariant and dropped.

  Scores St[m,n] = k8^T q8 — ONE fp8 DoubleRow matmul per 128-key chunk
     (contracts all 256 channels at the fp8 rate).
  The q8 quantization error couples to the output through cov(k,v); it is
  cancelled by a rank-256 correction accumulated straight into the
  attention output PSUM:  OT += dq^T (Wk^T Wv) * (NPIX*chat/16)  — the
  `om` input, with chat = E[P] estimated host-side by sampling.
  P = exp(St/16 + bqk) in bf16: Act-engine exp for 24/32 key chunks, the
  bf16 Schraudolph bit-trick on DVE for 8/32.
  OT = v^T P in bf16; rowsums via Pool-engine accumulators (SBUF only —
  GPSIMD cannot touch PSUM) + a ones-matmul over partitions.
"""

import sys

sys.path.insert(0, "/opt/trn_rl_repo")

import numpy as np
import concourse.bacc as bacc
import concourse.mybir as mybir
import concourse.tile as tile
from concourse.bass_utils import run_bass_kernel_spmd

F32 = mybir.dt.float32
F32R = mybir.dt.float32r
FP8 = mybir.dt.float8e4
BF16 = mybir.dt.bfloat16
I16 = mybir.dt.int16
AF = mybir.ActivationFunctionType
ALU = mybir.AluOpType
DRM = mybir.MatmulPerfMode.DoubleRow

B = 8
C = 256
NPIX = 4096
NT = NPIX // 512
M = NPIX // 128
PAIRS = M // 2
SCALE = 1.0 / 16.0
LAG_P = 8
# exp engine per key-chunk (32 halves/nt): a=Act exp, d=DVE bf16-Schraudolph
# (bits16 = y*128*log2e + 126.946*128, truncating convert)
EPAT = ["a", "a", "a", "d", "a", "a", "a", "d",
        "a", "a", "d", "a", "a", "a", "a", "d"] * 2
A16 = 128.0 * np.log2(np.e)
B16 = 126.946 * 128.0

_CACHE = {}


def _build():
    nc = bacc.Bacc("TRN2", num_swdge_queues=4)
    x_d = nc.declare_dram_parameter("x", [C, NPIX], F32, isOutput=False)
    wq_d = nc.declare_dram_parameter("wq_t", [C, C], F32, isOutput=False)
    wk_d = nc.declare_dram_parameter("wk_t", [C, C], F32, isOutput=False)
    wv_d = nc.declare_dram_parameter("wv_t", [C, C], F32, isOutput=False)
    om_d = nc.declare_dram_parameter("om", [C, C], F32, isOutput=False)
    bq_d = nc.declare_dram_parameter("bq", [C, 1], F32, isOutput=False)
    bv_d = nc.declare_dram_parameter("bv", [1, C], F32, isOutput=False)
    out_d = nc.declare_dram_parameter("out", [C, NPIX], F32, isOutput=True)

    with tile.TileContext(nc) as tc:
        with (
            tc.tile_pool(name="big", bufs=1) as big,
            tc.tile_pool(name="small", bufs=2) as small,
            tc.tile_pool(name="ptp", bufs=LAG_P + 3) as ptp,
            tc.tile_pool(name="outp", bufs=4) as outp,
            tc.tile_pool(name="psS", bufs=2, space="PSUM") as psS,
            tc.tile_pool(name="psO", bufs=1, space="PSUM") as psO,
            tc.tile_pool(name="psW", bufs=1, space="PSUM") as psW,
            tc.tile_pool(name="psR", bufs=1, space="PSUM") as psR,
        ):
            # ---- input DMAs (need-ordered) ----
            w_r = {}
            for nm in ("q", "k", "v", "o"):
                w_r[nm] = [
                    big.tile([128, C], F32R, name=f"w{nm}_r{i}") for i in range(2)
                ]
            for nm, wd in (("q", wq_d), ("k", wk_d)):
                for i in range(2):
                    nc.gpsimd.dma_start(
                        out=w_r[nm][i], in_=wd[i * 128 : (i + 1) * 128, :]
                    )
            x_r = [big.tile([128, NPIX], F32R, name=f"x_r{i}") for i in range(2)]
            for j in range(2):
                lo, hi = j * 512, (j + 1) * 512
                for i in range(2):
                    nc.gpsimd.dma_start(
                        out=x_r[i][:, lo:hi], in_=x_d[i * 128 : (i + 1) * 128, lo:hi]
                    )
            for nm, wd in (("v", wv_d), ("o", om_d)):
                for i in range(2):
                    nc.gpsimd.dma_start(
                        out=w_r[nm][i], in_=wd[i * 128 : (i + 1) * 128, :]
                    )
            for j in range(2, 8):
                lo, hi = j * 512, (j + 1) * 512
                for i in range(2):
                    nc.gpsimd.dma_start(
                        out=x_r[i][:, lo:hi], in_=x_d[i * 128 : (i + 1) * 128, lo:hi]
                    )
            bq8 = big.tile([128, 2, 1], FP8, name="bq8")
            for i in range(2):
                nc.gpsimd.dma_start(
                    out=bq8[:, i, :], in_=bq_d[i * 128 : (i + 1) * 128, :]
                )
            bv_f = big.tile([1, C], F32, name="bv_f")
            nc.sync.dma_start(out=bv_f, in_=bv_d[:, :])
            bv_r = big.tile([1, C], F32R, name="bv_r")
            nc.vector.tensor_copy(bv_r, bv_f)

            ones_f = big.tile([128, 1], F32, name="ones_f")
            nc.vector.memset(ones_f, 1.0)
            ones_rf = big.tile([1, 128], F32, name="ones_rf")
            nc.vector.memset(ones_rf, 1.0)
            ones_colr = big.tile([1, 128], F32R, name="ones_colr")
            nc.vector.tensor_copy(ones_colr, ones_rf)
            ones_row = big.tile([1, 128], F32R, name="ones_row")
            nc.vector.tensor_copy(ones_row, ones_rf)
            ones_col = big.tile([128, 1], F32R, name="ones_col")
            nc.vector.tensor_copy(ones_col, ones_f)

            # ---- PE warmup (p-state ramp), baseline pattern ----
            warm_f = small.tile([128, 256], F32, name="warm_f", tag="warm_f")
            nc.vector.memset(warm_f, 1.0)
            warm_r = small.tile([128, 256], F32R, name="warm_r", tag="warm_r")
            nc.vector.tensor_copy(warm_r, warm_f)
            warm_ps = psR.tile([1, 256], F32, name="warm_ps", tag="psR")
            for _ in range(48):
                nc.tensor.matmul(
                    warm_ps, ones_col, warm_r, start=True, stop=True,
                    skip_group_check=True,
                )

            # ---- Q/K projections (f32r), nt-major pair layout ----
            q8 = big.tile([128, NT, 2, 512], FP8, name="q8")
            dq_r = big.tile([128, NT, 2, 512], F32R, name="dq_r")
            k8 = big.tile([128, NT, 2, 512], FP8, name="k8")
            for nt in range(NT):
                sl = slice(nt * 512, (nt + 1) * 512)
                for wkey in ("q", "k"):
                    ps = psS.tile([128, 1024], F32, name="ps_proj", tag="pair")
                    for o in range(2):
                        for i in range(2):
                            nc.tensor.matmul(
                                ps[:, o * 512 : (o + 1) * 512],
                                w_r[wkey][i][:, o * 128 : (o + 1) * 128],
                                x_r[i][:, sl],
                                start=(i == 0),
                                stop=(i == 1),
                            )
                    if wkey == "k":
                        nc.scalar.activation(k8[:, nt], ps, AF.Copy)
                    else:
                        if nt % 2 == 0:
                            nc.vector.tensor_copy(q8[:, nt], ps)
                        else:
                            nc.scalar.activation(q8[:, nt], ps, AF.Copy)
                        nc.vector.tensor_tensor(
                            dq_r[:, nt], ps, q8[:, nt], ALU.subtract
                        )

            # ---- bqk[m] = (bq . k8_m) (exp bias), tiny DR matmuls ----
            def k8_lhsT(m):
                return k8[:, m // 4, :, (m % 4) * 128 : (m % 4 + 1) * 128]

            bqk_ps = psR.tile([128, M], F32, name="bqk_ps", tag="psR")
            for m in range(M):
                nc.tensor.matmul(
                    bqk_ps[:, m : m + 1], k8_lhsT(m), bq8,
                    start=True, stop=True, perf_mode=DRM,
                )
            s2a = big.tile([128, M], F32, name="s2a")
            nc.vector.tensor_scalar(s2a, bqk_ps, SCALE, None, ALU.mult)
            s2d = big.tile([128, M], F32, name="s2d")
            nc.vector.tensor_scalar(s2d, bqk_ps, A16 * SCALE, B16,
                                    ALU.mult, ALU.add)

            # ---- V projection (f32r, bias via ones-matmul) -> bf16 ----
            v_bf = big.tile([128, M, C], BF16, name="v_bf")
            for g in range(8):  # 4 m-chunks per psum pair tile
                ps = psS.tile([128, 1024], F32, name="ps_v", tag="pair")
                for j in range(4):
                    m = 4 * g + j
                    po = ps[:, j * 256 : (j + 1) * 256]
                    for i in range(2):
                        nc.tensor.matmul(
                            po,
                            x_r[i][:, m * 128 : (m + 1) * 128],
                            w_r["v"][i],
                            start=(i == 0),
                            stop=False,
                        )
                    nc.tensor.matmul(po, ones_colr, bv_r, start=False, stop=True)
                for j in range(2):  # two [128,512] pair copies
                    m = 4 * g + 2 * j
                    po = ps[:, j * 512 : (j + 1) * 512]
                    if j == 0:
                        nc.scalar.activation(v_bf[:, m : m + 2, :], po, AF.Copy)
                    else:
                        nc.vector.tensor_copy(v_bf[:, m : m + 2, :], po)

            # ---- attention ----
            def drain_a(state):
                """rowsum -> reciprocal (off the PE critical path)."""
                p_ot0, p_ot1, p_accs, p_nt = state
                rs = psW.tile([1, 512], F32, name="rs", tag="rs")
                for ai, acc in enumerate(p_accs):
                    nc.tensor.matmul(rs, ones_col, acc,
                                     start=(ai == 0), stop=(ai == 3))
                rinv = small.tile([1, 512], F32, name="rinv", tag="rinv")
                nc.vector.reciprocal_approx_fast(rinv, rs)
                rinv_r = small.tile([1, 512], F32R, name="rinv_r", tag="rinv_r")
                nc.vector.tensor_copy(rinv_r, rinv)
                return rinv_r

            def drain_b(state, rinv_r):
                """broadcast 1/rowsum, normalize, store."""
                p_ot0, p_ot1, p_accs, p_nt = state
                rb = psR.tile([128, 512], F32, name="rb", tag="psR")
                nc.tensor.matmul(rb, ones_row, rinv_r, start=True, stop=True)
                rb_sb = small.tile([128, 512], F32, name="rb_sb", tag="rb_sb")
                nc.scalar.activation(rb_sb, rb, AF.Copy)
                psl = slice(p_nt * 512, (p_nt + 1) * 512)
                for o, ot in enumerate((p_ot0, p_ot1)):
                    osb = outp.tile([128, 512], F32, name="osb", tag="osb")
                    nc.vector.tensor_tensor(osb, ot, rb_sb, ALU.mult)
                    nc.sync.dma_start(
                        out=out_d[o * 128 : (o + 1) * 128, psl], in_=osb
                    )

            prev = None
            rinv_prev = None
            for nt in range(NT):
                ot0 = psO.tile([128, 512], F32, name="ot0", tag="ot0")
                ot1 = psO.tile([128, 512], F32, name="ot1", tag="ot1")
                accs = [
                    small.tile([128, 512], F32R, name=f"acc{ai}", tag=f"acc{ai}")
                    for ai in range(4)
                ]
                pts = {}
                for pp in range(PAIRS + LAG_P):
                    if pp < PAIRS:
                        st = psS.tile([128, 1024], F32, name="st", tag="pair")
                        pt = ptp.tile([128, 2, 512], BF16, name="pt")
                        for h in range(2):
                            m = 2 * pp + h
                            half = st[:, h * 512 : (h + 1) * 512]
                            nc.tensor.matmul(
                                half, k8_lhsT(m), q8[:, nt],
                                start=True, stop=True, perf_mode=DRM,
                            )
                            if EPAT[m] == "a":
                                nc.scalar.activation(
                                    pt[:, h, :], half, AF.Exp,
                                    bias=s2a[:, m : m + 1], scale=SCALE,
                                )
                            else:
                                nc.vector.tensor_scalar(
                                    pt[:, h, :].bitcast(I16), half,
                                    A16 * SCALE, s2d[:, m : m + 1],
                                    ALU.mult, ALU.add,
                                )
                            # rowsum accs: chains of 8; DVE h==0, Pool h==1
                            eacc = nc.vector if h == 0 else nc.gpsimd
                            acc = accs[2 * h + pp % 2]
                            if pp < 2:
                                eacc.tensor_copy(acc, pt[:, h, :])
                            else:
                                eacc.tensor_tensor(
                                    acc, acc, pt[:, h, :], ALU.add
                                )
                        pts[pp] = pt
                    if prev is not None and pp == 3:
                        rinv_prev = drain_a(prev)
                    if prev is not None and pp == 6:
                        drain_b(prev, rinv_prev)
                    if pp >= LAG_P:
                        p = pp - LAG_P
                        pt = pts.pop(p)
                        for h in range(2):
                            m = 2 * p + h
                            rhs = pt[:, h, :]
                            first = (p == 0 and h == 0)
                            last = (p == PAIRS - 1 and h == 1)
                            nc.tensor.matmul(
                                ot0, v_bf[:, m, 0:128], rhs,
                                start=first, stop=last,
                            )
                            nc.tensor.matmul(
                                ot1, v_bf[:, m, 128:256], rhs,
                                start=first, stop=last,
                            )
                        # omega correction joins each psum group mid-way
                        if p == 2:
                            for o, ot in enumerate((ot0, ot1)):
                                for i in range(2):
                                    nc.tensor.matmul(
                                        ot,
                                        w_r["o"][i][:, o * 128 : (o + 1) * 128],
                                        dq_r[:, nt, i, :],
                                        start=False,
                                        stop=False,
                                    )
                prev = (ot0, ot1, accs, nt)

            rinv_prev = drain_a(prev)
            drain_b(prev, rinv_prev)

    nc.compile()
    return nc


def _get_nc():
    if "nc" not in _CACHE:
        _CACHE["nc"] = _build()
    return _CACHE["nc"]


def _host_prep(x, wq, wk, wv, bq, bv):
    """Per-core input maps incl. the omega correction matrix."""
    Wq = np.ascontiguousarray(wq.T)
    Wk = np.ascontiguousarray(wk.T)
    Wv = np.ascontiguousarray(wv.T)
    om_base = (wk @ wv.T).astype(np.float32)  # = Wk.T @ Wv
    rng = np.random.default_rng(12345)
    shared = {
        "wq_t": Wq,
        "wk_t": Wk,
        "wv_t": Wv,
        "bq": np.ascontiguousarray(bq.reshape(C, 1)),
        "bv": np.ascontiguousarray(bv.reshape(1, C)),
    }
    maps = []
    for b in range(B):
        xb = np.ascontiguousarray(x[b].reshape(C, NPIX))
        qi = rng.choice(NPIX, 128, replace=False)
        ki = rng.choice(NPIX, 256, replace=False)
        qs = xb[:, qi].T @ Wq  # unbiased q; bias enters via bqk
        ks = xb[:, ki].T @ Wk
        bqks = ks @ bq.reshape(-1)
        y = (qs @ ks.T) * SCALE + bqks[None, :] * SCALE
        chat = float(np.exp(y).mean())
        om = np.ascontiguousarray(om_base * (NPIX * chat / 16.0))
        maps.append({"x": xb, "om": om, **shared})
    return maps


def kernel(x, wq, wk, wv, bq, bk, bv):
    x = np.asarray(x, dtype=np.float32)
    wq = np.asarray(wq, dtype=np.float32)
    wk = np.asarray(wk, dtype=np.float32)
    wv = np.asarray(wv, dtype=np.float32)
    bq = np.asarray(bq, dtype=np.float32)
    bv = np.asarray(bv, dtype=np.float32)
    nc = _get_nc()
    res = run_bass_kernel_spmd(
        nc, _host_prep(x, wq, wk, wv, bq, bv), core_ids=list(range(B))
    )
    out = np.stack([res.results[b]["out"] for b in range(B)])
    return out.reshape(B, C, 64, 64)
